# revision 38
# baseline (speedup 1.0000x reference)
"""Deformable cross-attention Trainium2 kernel (Bass/Tile), SPMD over 8 cores.

Sharding: data-parallel over batch B=8 — one batch element per NeuronCore.

Per core, for each chunk of 128 queries:
  1. PE: transpose SADQ chunk, project to offsets/attn-logits/base (one fused
     [256 x 98] matmul).
  2. DVE/ACT: softmax over the 4 points per head, bilinear corner weights
     (fp32 magic-constant floor), validity masks, per-pair coefficients, and
     flat gather indices.  Each sample point becomes two "pair" gathers
     (rows y0 and y1), each pulling 2 adjacent pixels (512 floats) so the two
     x-corners ride one descriptor.  Edge cases (x0 == -1 left swap,
     g == 9999 bottom-right overflow) are handled branchlessly by swapping
     the pair's two coefficients.
  3. PE "repl-transpose": one transpose-matmul with a 0-stride broadcast AP
     turns [128q, 16slot] coefficient/index slices into [128, 128] tiles
     replicated 8x across partition groups — exactly the layout dma_gather's
     index table needs, and (band-masked) the rhs for the sampling matmuls.
  4. GPSIMD dma_gather: 2048 indices x 2KiB pairs per call, 4 calls/chunk.
  5. PE: sampling matmuls — gathered pairs are the stationary operand, the
     band-masked coefficient tiles the moving operand, accumulating agg.T
     [256 x 128] directly in PSUM (weighted corner sum + per-query reduction
     + head-mean fused into the contraction).
  6. PE: agg.T @ Wp -> out chunk, add bias, DMA out.
"""
import numpy as np
from contextlib import ExitStack

import concourse.bass as bass
import concourse.bacc as bacc
import concourse.mybir as mybir
import concourse.tile as tile
from concourse.alu_op_type import AluOpType as Op
from concourse.bass_utils import run_bass_kernel_spmd

FP = mybir.dt.float32
I16 = mybir.dt.int16
AF = mybir.ActivationFunctionType

M, D, GR = 1024, 256, 10000     # queries, dim, grid rows (H*W)
CHUNK = 128                     # queries per chunk
NCHUNK = M // CHUNK
MAGIC = 12582912.0              # 1.5 * 2**23: fp32 round-to-int constant
NIDX_CALL = 512                 # gather indices per dma_gather call





def build_kernel(ctx: ExitStack, tc: tile.TileContext, outs, ins, nchunk=NCHUNK,
                 dbg=None, repeat=1, no_gather=False):
    nc = tc.nc
    out_d = outs[0]

    seen_dbg = set()

    def dbg_out(name, ap, c):
        if dbg is None or c != 0 or name in seen_dbg:
            return
        seen_dbg.add(name)
        t = nc.dram_tensor(f"DBG_{name}", list(ap.shape), ap.dtype,
                           kind="ExternalOutput")
        nc.sync.dma_start(t.ap(), ap)
        dbg.append(name)
    sadq, e_d, wo, bo, wa, ba, wr, br, wp, bp = ins

    const = ctx.enter_context(tc.tile_pool(name="const", bufs=1))
    qpool = ctx.enter_context(tc.tile_pool(name="q", bufs=2))
    tpool = ctx.enter_context(tc.tile_pool(name="t", bufs=2))
    cpool = ctx.enter_context(tc.tile_pool(name="cm", bufs=2))
    gpool = ctx.enter_context(tc.tile_pool(name="g", bufs=2))
    opool = ctx.enter_context(tc.tile_pool(name="o", bufs=2))
    ppool = ctx.enter_context(tc.tile_pool(name="ps", bufs=2, space="PSUM"))
    pfront = ctx.enter_context(tc.tile_pool(name="pf", bufs=1, space="PSUM"))
    pagg = ctx.enter_context(tc.tile_pool(name="pagg", bufs=2, space="PSUM"))
    pout = ctx.enter_context(tc.tile_pool(name="pout", bufs=1, space="PSUM"))

    # ---- constants ----------------------------------------------------------
    ones1 = const.tile([1, 128], FP, tag="ones1")
    nc.vector.memset(ones1[:], 1.0)
    ident = const.tile([128, 128], FP, tag="ident")
    nc.vector.memset(ident[:], 1.0)
    nc.gpsimd.affine_select(ident[:], ident[:], [[-1, 128]], Op.is_equal, 0.0,
                            base=0, channel_multiplier=1)
    band = const.tile([128, 128], FP, tag="band")
    nc.vector.memset(band[:], 1.0)
    # band[i, 8r+dc] = 1 iff i//16 == dc:  0 <= i - 16*dc <= 15
    bview = band[:].rearrange("p (r c) -> p r c", c=8)
    nc.gpsimd.affine_select(bview, bview, [[0, 16], [-16, 8]], Op.is_ge, 0.0,
                            base=0, channel_multiplier=1)
    # i - 16*dc <= 15  <=>  15 - i + 16*dc >= 0
    nc.gpsimd.affine_select(bview, bview, [[0, 16], [16, 8]], Op.is_ge, 0.0,
                            base=15, channel_multiplier=-1)
    # rep4[sg][k, i] = 1 iff k == 16*sg + i%16  (k in [0,64), i in [0,128))
    rep4 = []
    for sg in range(4):
        t = const.tile([64, 128], FP, tag=f"rep{sg}")
        nc.vector.memset(t[:], 1.0)
        tv = t[:].rearrange("p (a b) -> p a b", b=16)
        nc.gpsimd.affine_select(tv, tv, [[0, 8], [-1, 16]], Op.is_equal, 0.0,
                                base=-16 * sg, channel_multiplier=1)
        rep4.append(t)

    # fused projection weights [256, 98] = Wo | Wa | Wr, two 128-row chunks
    woar = []
    for dc in range(2):
        t = const.tile([128, 98], FP, tag=f"woar{dc}")
        r0 = 128 * dc
        nc.sync.dma_start(t[:, 0:64], wo[r0:r0 + 128, :])
        nc.sync.dma_start(t[:, 64:96], wa[r0:r0 + 128, :])
        nc.sync.dma_start(t[:, 96:98], wr[r0:r0 + 128, :])
        woar.append(t)
    wpt = []
    for dc in range(2):
        t = const.tile([128, 256], FP, tag=f"wp{dc}")
        nc.sync.dma_start(t[:], wp[128 * dc:128 * dc + 128, :])
        wpt.append(t)

    # biases, replicated across partitions via k=1 matmul broadcast
    brow = const.tile([1, 98], FP, tag="brow")
    nc.sync.dma_start(brow[:, 0:64], bo[None, :])
    nc.sync.dma_start(brow[:, 64:96], ba[None, :])
    nc.sync.dma_start(brow[:, 96:98], br[None, :])
    bprow = const.tile([1, 256], FP, tag="bprow")
    nc.sync.dma_start(bprow[:], bp[None, :])
    pb = pfront.tile([128, 256], FP, tag="pf", name="pb")
    nc.tensor.matmul(pb[:, 0:98], ones1[:], brow[:], start=True, stop=True)
    bias98 = const.tile([128, 98], FP, tag="bias98")
    nc.vector.tensor_copy(bias98[:], pb[:, 0:98])
    pb2 = pfront.tile([128, 256], FP, tag="pf", name="pb2")
    nc.tensor.matmul(pb2[:], ones1[:], bprow[:], start=True, stop=True)
    bp256 = const.tile([128, 256], FP, tag="bp256")
    nc.vector.tensor_copy(bp256[:], pb2[:])

    # gather source: overlapping-window AP over E rows: idx g -> rows [g, g+2)
    e_win = bass.AP(e_d.tensor, 0, [[256, GR - 1], [1, 512]])

    # ---- per-chunk pipeline -------------------------------------------------
    for c in [ci for _ in range(repeat) for ci in range(nchunk)]:
        c0 = c * CHUNK

        qn = qpool.tile([128, 256], FP, tag="qn")
        nc.sync.dma_start(qn[:], sadq[c0:c0 + CHUNK, :])
        qt = []
        for dc in range(2):
            ptr = ppool.tile([128, 128], FP, tag="ptr")
            nc.tensor.transpose(ptr[:], qn[:, 128 * dc:128 * dc + 128], ident[:])
            t = qpool.tile([128, 128], FP, tag=f"qt{dc}")
            nc.vector.tensor_copy(t[:], ptr[:])
            qt.append(t)

        pft = pfront.tile([128, 256], FP, tag="pf", name="pf")
        for dc in range(2):
            nc.tensor.matmul(pft[:, 0:98], qt[dc][:], woar[dc][:],
                             start=(dc == 0), stop=(dc == 1))
        f = tpool.tile([128, 98], FP, tag="f")
        nc.vector.tensor_tensor(f[:], pft[:, 0:98], bias98[:], Op.add)
        dbg_out("f", f[:], c)

        # softmax over points (groups of 4), fold 1/NHEAD
        e1 = tpool.tile([128, 32], FP, tag="e1")
        nc.scalar.activation(e1[:], f[:, 64:96], AF.Exp)
        s = tpool.tile([128, 8], FP, tag="s")
        nc.vector.reduce_sum(s[:], e1[:].rearrange("p (h x) -> p h x", x=4),
                             axis=mybir.AxisListType.X)
        r = tpool.tile([128, 8], FP, tag="r")
        nc.vector.reciprocal(r[:], s[:])
        r2 = tpool.tile([128, 8], FP, tag="r2")
        nc.vector.tensor_scalar_mul(r2[:], r[:], 0.125)
        aw = tpool.tile([128, 32], FP, tag="aw")
        for p in range(4):
            nc.vector.tensor_tensor(aw[:, p::4], e1[:, p::4], r2[:], Op.mult)
        dbg_out("aw", aw[:], c)

        # grid coords (ixp = ix - 0.5 so magic-round gives floor(ix))
        sx = tpool.tile([128, 1], FP, tag="sx")
        nc.vector.tensor_scalar(sx[:], f[:, 96:97], 50.0, 49.0, Op.mult, Op.add)
        sy = tpool.tile([128, 1], FP, tag="sy")
        nc.vector.tensor_scalar(sy[:], f[:, 97:98], 50.0, 49.0, Op.mult, Op.add)
        ixp = tpool.tile([128, 32], FP, tag="ixp")
        nc.vector.tensor_scalar(ixp[:], f[:, 0:64:2], 5.0, sx[:], Op.mult, Op.add)
        iyp = tpool.tile([128, 32], FP, tag="iyp")
        nc.vector.tensor_scalar(iyp[:], f[:, 1:64:2], 5.0, sy[:], Op.mult, Op.add)

        x0 = tpool.tile([128, 32], FP, tag="x0")
        nc.vector.tensor_scalar(x0[:], ixp[:], MAGIC, -MAGIC, Op.add, Op.add)
        y0 = tpool.tile([128, 32], FP, tag="y0")
        nc.vector.tensor_scalar(y0[:], iyp[:], MAGIC, -MAGIC, Op.add, Op.add)

        wx1 = tpool.tile([128, 32], FP, tag="wx1")
        nc.vector.scalar_tensor_tensor(wx1[:], ixp[:], 0.5, x0[:], Op.add, Op.subtract)
        wy1 = tpool.tile([128, 32], FP, tag="wy1")
        nc.vector.scalar_tensor_tensor(wy1[:], iyp[:], 0.5, y0[:], Op.add, Op.subtract)
        wx0 = tpool.tile([128, 32], FP, tag="wx0")
        nc.vector.tensor_scalar(wx0[:], wx1[:], -1.0, 1.0, Op.mult, Op.add)
        wy0 = tpool.tile([128, 32], FP, tag="wy0")
        nc.vector.tensor_scalar(wy0[:], wy1[:], -1.0, 1.0, Op.mult, Op.add)

        # validity-folded corner weights: w *= (v >= lo) * (v <= hi)
        def masked(w, v, lo, hi, tag):
            t1 = tpool.tile([128, 32], FP, tag=tag + "a")
            nc.vector.scalar_tensor_tensor(t1[:], v[:], float(lo), w[:], Op.is_ge, Op.mult)
            t2 = tpool.tile([128, 32], FP, tag=tag + "b")
            nc.vector.scalar_tensor_tensor(t2[:], v[:], float(hi), t1[:], Op.is_le, Op.mult)
            return t2

        wx0v = masked(wx0, x0, 0, 99, "wx0v")
        wx1v = masked(wx1, x0, -1, 98, "wx1v")
        wy0v = masked(wy0, y0, 0, 99, "wy0v")
        wy1v = masked(wy1, y0, -1, 98, "wy1v")
        dbg_out("x0", x0[:], c)
        dbg_out("y0", y0[:], c)
        dbg_out("wx1", wx1[:], c)
        dbg_out("wx0v", wx0v[:], c)
        dbg_out("wx1v", wx1v[:], c)
        dbg_out("wy0v", wy0v[:], c)
        dbg_out("wy1v", wy1v[:], c)

        t0 = tpool.tile([128, 32], FP, tag="t0")
        nc.vector.tensor_tensor(t0[:], aw[:], wy0v[:], Op.mult)
        t1_ = tpool.tile([128, 32], FP, tag="t1")
        nc.vector.tensor_tensor(t1_[:], aw[:], wy1v[:], Op.mult)

        cl = tpool.tile([128, 64], FP, tag="cl")
        cr = tpool.tile([128, 64], FP, tag="cr")
        gidx = tpool.tile([128, 64], FP, tag="gidx")

        c00 = tpool.tile([128, 32], FP, tag="c00", name="c00")
        c01 = tpool.tile([128, 32], FP, tag="c01", name="c01")
        c10 = tpool.tile([128, 32], FP, tag="c10", name="c10")
        c11 = tpool.tile([128, 32], FP, tag="c11", name="c11")
        nc.vector.tensor_tensor(c00[:], t0[:], wx0v[:], Op.mult)
        nc.vector.tensor_tensor(c01[:], t0[:], wx1v[:], Op.mult)
        nc.vector.tensor_tensor(c10[:], t1_[:], wx0v[:], Op.mult)
        nc.vector.tensor_tensor(c11[:], t1_[:], wx1v[:], Op.mult)

        x0c = tpool.tile([128, 32], FP, tag="x0c")
        nc.vector.tensor_scalar(x0c[:], x0[:], -1.0, 99.0, Op.max, Op.min)
        y0c = tpool.tile([128, 32], FP, tag="y0c")
        nc.vector.tensor_scalar(y0c[:], y0[:], 0.0, 99.0, Op.max, Op.min)
        y1c = tpool.tile([128, 32], FP, tag="y1c")
        nc.vector.tensor_scalar(y1c[:], y0[:], 1.0, 0.0, Op.add, Op.max)
        nc.vector.tensor_scalar_min(y1c[:], y1c[:], 99.0)

        m = tpool.tile([128, 32], FP, tag="m")
        nc.vector.tensor_scalar(m[:], x0c[:], -1.0, None, Op.is_equal)
        nm = tpool.tile([128, 32], FP, tag="nm")
        nc.vector.tensor_scalar(nm[:], m[:], -1.0, 1.0, Op.mult, Op.add)

        # left-edge swap into slot-layout halves of CL/CR
        for (ca, cb, half) in ((c00, c01, 0), (c10, c11, 1)):
            dlt = tpool.tile([128, 32], FP, tag=f"dlt{half}")
            nc.vector.tensor_tensor(dlt[:], cb[:], ca[:], Op.subtract)
            md = tpool.tile([128, 32], FP, tag=f"md{half}")
            nc.vector.tensor_tensor(md[:], m[:], dlt[:], Op.mult)
            nc.vector.tensor_tensor(cl[:, 32 * half:32 * half + 32], md[:], ca[:], Op.add)
            nc.vector.tensor_tensor(cr[:, 32 * half:32 * half + 32], cb[:], nm[:], Op.mult)

        x0g = tpool.tile([128, 32], FP, tag="x0g")
        nc.vector.tensor_scalar_max(x0g[:], x0c[:], 0.0)
        for (yc, half) in ((y0c, 0), (y1c, 1)):
            gsl = gidx[:, 32 * half:32 * half + 32]
            nc.vector.scalar_tensor_tensor(gsl, yc[:], 100.0, x0g[:], Op.mult, Op.add)
            m2 = tpool.tile([128, 32], FP, tag=f"m2{half}")
            nc.vector.tensor_scalar(m2[:], gsl, 9999.0, None, Op.is_ge)
            nc.vector.tensor_tensor(gsl, gsl, m2[:], Op.subtract)
            clsl = cl[:, 32 * half:32 * half + 32]
            crsl = cr[:, 32 * half:32 * half + 32]
            mc = tpool.tile([128, 32], FP, tag=f"mc{half}")
            nc.vector.tensor_tensor(mc[:], m2[:], clsl, Op.mult)
            nc.vector.tensor_tensor(crsl, crsl, mc[:], Op.add)
            nm2 = tpool.tile([128, 32], FP, tag=f"nm2{half}")
            nc.vector.tensor_scalar(nm2[:], m2[:], -1.0, 1.0, Op.mult, Op.add)
            nc.vector.tensor_tensor(clsl, clsl, nm2[:], Op.mult)

        dbg_out("cl", cl[:], c)
        dbg_out("cr", cr[:], c)
        dbg_out("gidx", gidx[:], c)

        # ---- per-slot-group: repl-transpose, gather, sampling matmuls ------
        pa0 = pagg.tile([128, 128], FP, tag="pa0")
        pa1 = pagg.tile([128, 128], FP, tag="pa1")
        tt = cpool.tile([128, 512], I16, tag="tt")

        # transpose [128q, 64slot] -> [64, 128q] once per matrix
        t64 = {}
        for nm, src in (("g", gidx), ("l", cl), ("r", cr)):
            pt = ppool.tile([128, 128], FP, tag="ptr", name=f"pt64{nm}")
            nc.tensor.transpose(pt[:64, :], src[:], ident[:])
            t = tpool.tile([64, 128], FP, tag=f"t64{nm}", name=f"t64{nm}")
            nc.vector.tensor_copy(t[:], pt[:64, :])
            t64[nm] = t

        for sg in range(4):
            pgi = ppool.tile([128, 128], FP, tag="ptr")
            nc.tensor.matmul(pgi[:], rep4[sg][:], t64["g"][:], start=True, stop=True)
            nc.vector.tensor_copy(tt[:, 128 * sg:128 * sg + 128], pgi[:])

            pcl = ppool.tile([128, 128], FP, tag="ptr")
            nc.tensor.matmul(pcl[:], rep4[sg][:], t64["l"][:], start=True, stop=True)
            clm = cpool.tile([128, 128], FP, tag="clm")
            nc.vector.tensor_tensor(clm[:], pcl[:], band[:], Op.mult)

            pcr = ppool.tile([128, 128], FP, tag="ptr")
            nc.tensor.matmul(pcr[:], rep4[sg][:], t64["r"][:], start=True, stop=True)
            crm = cpool.tile([128, 128], FP, tag="crm")
            nc.vector.tensor_tensor(crm[:], pcr[:], band[:], Op.mult)

            dbg_out(f"clm{sg}", clm[:], c)
            dbg_out(f"crm{sg}", crm[:], c)

            bpc = NIDX_CALL // 128            # k-blocks per gather call
            for sub in range(16 // bpc):
                gt = gpool.tile([128, bpc, 512], FP, tag="gt")
                icol = 128 * sg + (NIDX_CALL // 16) * sub
                if no_gather:
                    nc.gpsimd.memset(gt[:], 0.25)
                else:
                    nc.gpsimd.dma_gather(
                        gt[:], e_win, tt[:, icol:icol + NIDX_CALL // 16],
                        NIDX_CALL, NIDX_CALL, 512, elem_step=256)
                if sub == 0:
                    dbg_out(f"gt{sg}", gt[:], c)

                for j in range(bpc):
                    r = bpc * sub + j
                    rs = slice(8 * r, 8 * r + 8)
                    for li, (lo, co) in enumerate(((0, clm), (256, crm))):
                        st = (sg == 0 and r == 0 and li == 0)
                        sp = (sg == 3 and r == 15 and li == 1)
                        nc.tensor.matmul(pa0[:, rs], gt[:, j, lo:lo + 128],
                                         co[:, rs], start=st, stop=sp,
                                         skip_group_check=True)
                        nc.tensor.matmul(pa1[:, rs], gt[:, j, lo + 128:lo + 256],
                                         co[:, rs], start=st, stop=sp,
                                         skip_group_check=True)

        # ---- output projection ---------------------------------------------
        po = pout.tile([128, 256], FP, tag="po")
        for dc, pa in enumerate((pa0, pa1)):
            ags = opool.tile([128, 128], FP, tag=f"ags{dc}")
            nc.vector.tensor_copy(ags[:], pa[:])
            dbg_out(f"aggT{dc}", ags[:], c)
            nc.tensor.matmul(po[:], ags[:], wpt[dc][:],
                             start=(dc == 0), stop=(dc == 1))
        ot = opool.tile([128, 256], FP, tag="ot")
        nc.vector.tensor_tensor(ot[:], po[:], bp256[:], Op.add)
        nc.sync.dma_start(out_d[c0:c0 + CHUNK, :], ot[:])


def build_nc(nchunk=NCHUNK, dbg=None, repeat=1, no_gather=False, num_devices=8):
    nc = bacc.Bacc("TRN2", target_bir_lowering=False, debug=False,
                   enable_asserts=False, num_devices=num_devices)
    ins = [
        nc.dram_tensor("SADQ", [M, D], FP, kind="ExternalInput").ap(),
        nc.dram_tensor("E", [GR, D], FP, kind="ExternalInput").ap(),
        nc.dram_tensor("Wo", [D, 64], FP, kind="ExternalInput").ap(),
        nc.dram_tensor("bo", [64], FP, kind="ExternalInput").ap(),
        nc.dram_tensor("Wa", [D, 32], FP, kind="ExternalInput").ap(),
        nc.dram_tensor("ba", [32], FP, kind="ExternalInput").ap(),
        nc.dram_tensor("Wr", [D, 2], FP, kind="ExternalInput").ap(),
        nc.dram_tensor("br", [2], FP, kind="ExternalInput").ap(),
        nc.dram_tensor("Wp", [D, D], FP, kind="ExternalInput").ap(),
        nc.dram_tensor("bp", [D], FP, kind="ExternalInput").ap(),
    ]
    outs = [nc.dram_tensor("OUT", [M, D], FP, kind="ExternalOutput").ap()]
    with tile.TileContext(nc) as tc:
        with ExitStack() as ctx:
            build_kernel(ctx, tc, outs, ins, nchunk=nchunk, dbg=dbg, repeat=repeat,
                         no_gather=no_gather)
    nc.compile()
    return nc


_NC = None


def kernel(**inputs) -> np.ndarray:
    global _NC
    if _NC is None:
        _NC = build_nc()
    sadq = np.ascontiguousarray(np.asarray(inputs["SADQ"], dtype=np.float32))
    e = np.ascontiguousarray(np.asarray(inputs["E"], dtype=np.float32))
    names = ["Wo", "bo", "Wa", "ba", "Wr", "br", "Wp", "bp"]
    shared = {n: np.ascontiguousarray(np.asarray(inputs[n], dtype=np.float32))
              for n in names}
    B = sadq.shape[0]
    in_maps = [dict(SADQ=sadq[b], E=e[b], **shared) for b in range(B)]
    res = run_bass_kernel_spmd(_NC, in_maps, list(range(B)))
    return np.stack([res.results[b]["OUT"] for b in range(B)])
